# revision 1
# baseline (speedup 1.0000x reference)
"""AttentionRNN Trainium2 kernel.

Data-parallel over batch: 8 cores x 8 sequences. Per core:
  phase 1: gather emb rows (indirect DMA), PE-transpose to E-major,
           xwt = x @ W_ih.T + (b_ih+b_hh)  -> DRAM, layout [(m p), (t b)] f32
  phase 2: 512-step scan h = tanh(xwt_t + W_hh @ h), UNMASKED
           (freezing h / zeroing outputs is equivalent to selecting
            outputs[b, len_b-1] as state and masking energies later).
           h kept transposed [H, B] as 8 tiles of [128, 8]; 32 steps per
           For_i iteration with a static history tile, DMA in/out per iter.
  phase 3: attention: state via one-hot-weighted reduce over outputs,
           u = Wa.T @ state, energy via PE with diagonal extraction,
           softmax, ctx via tensor_tensor_reduce, 2-layer MLP.
Output [16, 8] per core -> host assembles [64, 16] f32.
"""

import numpy as np
import ml_dtypes

B, T, E, H, V, C = 64, 512, 512, 1024, 50000, 16
NCORES = 8
BC = B // NCORES          # 8 sequences per core
MT = H // 128             # 8 h-tiles
ET = E // 128             # 4 e-tiles
RT = (BC * T) // 128      # 32 row-tiles of gathered x
U = 32                    # scan steps per For_i iteration
NIT = T // U              # 16 iterations

F32 = np.float32
BF16 = ml_dtypes.bfloat16


def _build():
    import concourse.bass as bass
    import concourse.mybir as mybir
    from concourse.tile import TileContext
    from concourse.masks import make_identity

    dt = mybir.dt
    AF = mybir.ActivationFunctionType
    ALU = mybir.AluOpType
    ds = bass.ds

    nc = bass.Bass()

    # ---- I/O ----
    emb_bf = nc.dram_tensor("emb_bf", [V, E], dt.bfloat16, kind="ExternalInput")
    tok_idx = nc.dram_tensor("tok_idx", [128, RT], dt.int32, kind="ExternalInput")
    whhT = nc.dram_tensor("whhT", [128, MT * H], dt.bfloat16, kind="ExternalInput")
    wihT = nc.dram_tensor("wihT", [128, ET * H], dt.bfloat16, kind="ExternalInput")
    wa_i = nc.dram_tensor("wa_i", [128, MT * H], dt.bfloat16, kind="ExternalInput")
    w1T = nc.dram_tensor("w1T", [128, MT * 64], dt.bfloat16, kind="ExternalInput")
    w2T = nc.dram_tensor("w2T", [64, C], dt.bfloat16, kind="ExternalInput")
    ball = nc.dram_tensor("ball", [128, MT], dt.float32, kind="ExternalInput")
    b1_i = nc.dram_tensor("b1_i", [64, 1], dt.float32, kind="ExternalInput")
    b2_i = nc.dram_tensor("b2_i", [C, 1], dt.float32, kind="ExternalInput")
    amask = nc.dram_tensor("amask", [1, BC * T], dt.float32, kind="ExternalInput")
    sel_i = nc.dram_tensor("sel_i", [1, BC * T], dt.bfloat16, kind="ExternalInput")
    res_o = nc.dram_tensor("res", [C, BC], dt.float32, kind="ExternalOutput")

    # scratch DRAM
    xwt_d = nc.dram_tensor("xwt_d", [H, T * BC], dt.float32, kind="Internal")
    outs_d = nc.dram_tensor("outs_d", [H, BC * T + 2], dt.bfloat16, kind="Internal")
    # views: rows (m p), xwt cols (t b), outs cols (b t)
    xwt_v = xwt_d[:, :].rearrange("(m p) (t b) -> p m t b", p=128, b=BC)
    outs_wv = outs_d[:, 0:BC * T].rearrange("(m p) (b t) -> p m b t", p=128, t=T)

    with TileContext(nc) as tc:
        with tc.tile_pool(name="singles", bufs=1) as sing:
            # persistent SBUF
            whhT_sb = sing.tile([128, MT * H], dt.bfloat16, tag="whhT")
            nc.sync.dma_start(out=whhT_sb, in_=whhT[:, :])
            ball_sb = sing.tile([128, MT], dt.float32, tag="ball")
            nc.sync.dma_start(out=ball_sb, in_=ball[:, :])
            hstage = sing.tile([128, U * 64], dt.bfloat16, tag="hstage")
            nc.gpsimd.memset(hstage[:, :], 0.0)

            # ---------------- phase 1: gather + transpose + xW ----------------
            with (
                tc.tile_pool(name="p1", bufs=3) as p1,
                tc.tile_pool(name="p1ps", bufs=3, space="PSUM") as p1ps,
                tc.tile_pool(name="xts", bufs=1) as xts,
            ):
                wihT_sb = xts.tile([128, ET * H], dt.bfloat16, tag="wihT")
                nc.sync.dma_start(out=wihT_sb, in_=wihT[:, :])
                ident_bf = xts.tile([128, 128], dt.bfloat16, tag="ident")
                make_identity(nc, ident_bf[:, :])
                tok_sb = xts.tile([128, RT], dt.int32, tag="tok")
                nc.gpsimd.dma_start(out=tok_sb, in_=tok_idx[:, :])
                # x^T tiles: e-tile et holds [128 e, 4096 r]
                xT = [xts.tile([128, RT * 128], dt.bfloat16, tag=f"xT{et}",
                               name=f"xT{et}")
                      for et in range(ET)]
                for r in range(RT):
                    x_g = p1.tile([128, E], dt.bfloat16, tag="xg", bufs=RT)
                    nc.gpsimd.indirect_dma_start(
                        out=x_g[:, :], out_offset=None,
                        in_=emb_bf[:, :],
                        in_offset=bass.IndirectOffsetOnAxis(
                            ap=tok_sb[:, r:r + 1], axis=0),
                    )
                    for et in range(ET):
                        pst = p1ps.tile([128, 128], dt.bfloat16, tag="tr")
                        nc.tensor.transpose(
                            out=pst[:, :], in_=x_g[:, et * 128:(et + 1) * 128],
                            identity=ident_bf[:, :])
                        nc.vector.tensor_copy(
                            out=xT[et][:, r * 128:(r + 1) * 128], in_=pst[:, :])
                # xW: for each 512-col block of r and each m-tile
                for rb in range(BC):
                    for m in range(MT):
                        psx = p1ps.tile([128, 512], dt.float32, tag="xw")
                        for et in range(ET):
                            nc.tensor.matmul(
                                psx[:, :],
                                wihT_sb[:, et * H + m * 128: et * H + (m + 1) * 128],
                                xT[et][:, rb * 512:(rb + 1) * 512],
                                start=(et == 0), stop=(et == ET - 1))
                        xw_sb = p1.tile([128, 512], dt.float32, tag="xwsb")
                        nc.vector.tensor_copy(out=xw_sb[:, :], in_=psx[:, :])
                        nc.sync.dma_start(
                            out=xwt_v[:, m, rb * 64:(rb + 1) * 64, :],
                            in_=xw_sb[:, :].rearrange("p (t b) -> p t b", b=BC))

            # ---------------- phase 2: the scan ----------------
            with (
                tc.tile_pool(name="scps", bufs=1, space="PSUM") as scps,
                tc.tile_pool(name="scio", bufs=2) as scio,
            ):
                with tc.For_i(0, T, U, hint_engines=(mybir.EngineType.PE,)) as iv:
                    xwt_it = scio.tile([128, MT * U * BC], dt.float32, tag="xwt")
                    xwt_itv = xwt_it[:, :].rearrange(
                        "p (m t b) -> p m t b", m=MT, b=BC)
                    nc.sync.dma_start(
                        out=xwt_itv, in_=xwt_v[:, :, ds(iv, U), :])
                    hsv = hstage[:, :].rearrange("p (m b u) -> p m b u",
                                                 b=BC, u=U)
                    for u in range(U):
                        pu = (u - 1) % U
                        for m in range(MT):
                            ps = scps.tile([128, BC], dt.float32, tag=f"ps{m}")
                            for k in range(MT):
                                nc.tensor.matmul(
                                    ps[:, :],
                                    whhT_sb[:, (k * MT + m) * 128:
                                            (k * MT + m + 1) * 128],
                                    hsv[:, k, :, pu],
                                    start=(k == 0), stop=(k == MT - 1))
                            nc.vector.tensor_add(
                                out=ps[:, :], in0=ps[:, :], in1=xwt_itv[:, m, u, :])
                            nc.scalar.activation(
                                out=hsv[:, m, :, u], in_=ps[:, :], func=AF.Tanh,
                                bias=ball_sb[:, m:m + 1], scale=1.0)
                    for m in range(MT):
                        nc.sync.dma_start(
                            out=outs_wv[:, m, :, ds(iv, U)],
                            in_=hsv[:, m, :, :])

            # ---------------- phase 3: attention + MLP ----------------
            with (
                tc.tile_pool(name="at", bufs=2) as at,
                tc.tile_pool(name="atbig", bufs=1) as atbig,
                tc.tile_pool(name="atps", bufs=2, space="PSUM") as atps,
            ):
                outs_sb = atbig.tile([128, MT * BC * T], dt.bfloat16, tag="outs")
                outs_v = outs_sb[:, :].rearrange(
                    "p (m b t) -> p m b t", m=MT, b=BC)
                nc.sync.dma_start(
                    out=outs_v,
                    in_=outs_d[:, 0:BC * T].rearrange("(m p) (b t) -> p m b t",
                                                      p=128, t=T))
                wa_sb = atbig.tile([128, MT * H], dt.bfloat16, tag="wa")
                nc.sync.dma_start(out=wa_sb, in_=wa_i[:, :])

                # state[h, b] = sum_t outs * onehot(len_b - 1)
                sel_sb = at.tile([1, BC * T], dt.bfloat16, tag="sel")
                nc.sync.dma_start(out=sel_sb, in_=sel_i[:, :])
                ones1 = at.tile([1, 128], dt.bfloat16, tag="ones1")
                nc.gpsimd.memset(ones1[:, :], 1.0)
                selbc = atbig.tile([128, BC * T], dt.bfloat16, tag="selbc")
                for b in range(BC):
                    psb = atps.tile([128, T], dt.float32, tag="bc")
                    nc.tensor.matmul(
                        psb[:, :], ones1[:, :],
                        sel_sb[:, b * T:(b + 1) * T], start=True, stop=True)
                    nc.vector.tensor_copy(
                        out=selbc[:, b * T:(b + 1) * T], in_=psb[:, :])
                stateT_f = at.tile([128, MT * BC], dt.float32, tag="stateTf")
                for m in range(MT):
                    for b in range(BC):
                        sprod = at.tile([128, T], dt.bfloat16, tag="prod", bufs=3)
                        nc.vector.tensor_mul(
                            out=sprod[:, :], in0=outs_v[:, m, b, :],
                            in1=selbc[:, b * T:(b + 1) * T])
                        sprod2 = at.tile([128, T], dt.bfloat16, tag="prod2", bufs=3)
                        nc.scalar.activation(
                            out=sprod2[:, :], in_=sprod[:, :], func=AF.Copy,
                            accum_out=stateT_f[:, m * BC + b: m * BC + b + 1])
                stateT = at.tile([128, MT * BC], dt.bfloat16, tag="stateT")
                nc.vector.tensor_copy(out=stateT[:, :], in_=stateT_f[:, :])

                # u = Wa^T @ state  (uu[k,b]), lhsT = Wa tiles (j on part)
                uu_bf = at.tile([128, MT * BC], dt.bfloat16, tag="uu")
                for kt in range(MT):
                    psu = atps.tile([128, BC], dt.float32, tag="uups", bufs=1)
                    for jt in range(MT):
                        nc.tensor.matmul(
                            psu[:, :],
                            wa_sb[:, jt * H + kt * 128: jt * H + (kt + 1) * 128],
                            stateT[:, jt * BC:(jt + 1) * BC],
                            start=(jt == 0), stop=(jt == MT - 1))
                    nc.vector.tensor_copy(
                        out=uu_bf[:, kt * BC:(kt + 1) * BC], in_=psu[:, :])

                # energy/softmax/broadcast per batch row on partition 0
                amask_sb = at.tile([1, BC * T], dt.float32, tag="amask")
                nc.sync.dma_start(out=amask_sb, in_=amask[:, :])
                wbc = atbig.tile([128, BC * T], dt.bfloat16, tag="wbc")
                for b in range(BC):
                    pse = atps.tile([1, T], dt.float32, tag="en")
                    for kt in range(MT):
                        nc.tensor.matmul(
                            pse[:, :],
                            uu_bf[:, kt * BC + b: kt * BC + b + 1],
                            outs_v[:, kt, b, :],
                            start=(kt == 0), stop=(kt == MT - 1))
                    en_m = at.tile([1, T], dt.float32, tag="enm")
                    nc.vector.tensor_add(
                        out=en_m[:, :], in0=pse[:, :],
                        in1=amask_sb[:, b * T:(b + 1) * T])
                    mx8 = at.tile([1, 8], dt.float32, tag="mx8")
                    nc.vector.max(out=mx8[:, :], in_=en_m[:, :])
                    negmax = at.tile([1, 1], dt.float32, tag="negmax")
                    nc.vector.tensor_scalar_mul(negmax[:, :], mx8[:, 0:1], -1.0)
                    w_f = at.tile([1, T], dt.float32, tag="wf")
                    sumexp = at.tile([1, 1], dt.float32, tag="sumexp")
                    nc.scalar.activation(
                        out=w_f[:, :], in_=en_m[:, :], func=AF.Exp,
                        bias=negmax[:, 0:1], scale=1.0, accum_out=sumexp[:, :])
                    rec = at.tile([1, 1], dt.float32, tag="rec")
                    nc.vector.reciprocal(rec[:, :], sumexp[:, :])
                    w_bf = at.tile([1, T], dt.bfloat16, tag="wbf")
                    nc.vector.tensor_scalar_mul(w_bf[:, :], w_f[:, :], rec[:, 0:1])
                    psb2 = atps.tile([128, T], dt.float32, tag="bc")
                    nc.tensor.matmul(
                        psb2[:, :], ones1[:, :], w_bf[:, :],
                        start=True, stop=True)
                    nc.vector.tensor_copy(
                        out=wbc[:, b * T:(b + 1) * T], in_=psb2[:, :])

                # ctx^T[h, b] = sum_t outs * w  (DVE mult + ACT free-dim accum)
                ctxT = at.tile([128, MT * BC], dt.float32, tag="ctxT")
                for m in range(MT):
                    for b in range(BC):
                        prod = at.tile([128, T], dt.bfloat16, tag="prod", bufs=3)
                        nc.vector.tensor_mul(
                            out=prod[:, :], in0=outs_v[:, m, b, :],
                            in1=wbc[:, b * T:(b + 1) * T])
                        prod2 = at.tile([128, T], dt.bfloat16, tag="prod2", bufs=3)
                        nc.scalar.activation(
                            out=prod2[:, :], in_=prod[:, :], func=AF.Copy,
                            accum_out=ctxT[:, m * BC + b: m * BC + b + 1])
                ctxT_bf = at.tile([128, MT * BC], dt.bfloat16, tag="ctxTb")
                nc.vector.tensor_copy(out=ctxT_bf[:, :], in_=ctxT[:, :])

                # MLP
                w1T_sb = at.tile([128, MT * 64], dt.bfloat16, tag="w1T")
                nc.sync.dma_start(out=w1T_sb, in_=w1T[:, :])
                b1_sb = at.tile([64, 1], dt.float32, tag="b1")
                nc.sync.dma_start(out=b1_sb, in_=b1_i[:, :])
                psh = atps.tile([64, BC], dt.float32, tag="mlp1", bufs=1)
                for kt in range(MT):
                    nc.tensor.matmul(
                        psh[:, :],
                        w1T_sb[:, kt * 64:(kt + 1) * 64],
                        ctxT_bf[:, kt * BC:(kt + 1) * BC],
                        start=(kt == 0), stop=(kt == MT - 1))
                hddT = at.tile([64, BC], dt.bfloat16, tag="hddT")
                nc.scalar.activation(
                    out=hddT[:, :], in_=psh[:, :], func=AF.Relu,
                    bias=b1_sb[:, 0:1], scale=1.0)
                w2T_sb = at.tile([64, C], dt.bfloat16, tag="w2T")
                nc.sync.dma_start(out=w2T_sb, in_=w2T[:, :])
                b2_sb = at.tile([C, 1], dt.float32, tag="b2")
                nc.sync.dma_start(out=b2_sb, in_=b2_i[:, :])
                pso = atps.tile([C, BC], dt.float32, tag="mlp2", bufs=1)
                nc.tensor.matmul(pso[:, :], w2T_sb[:, :], hddT[:, :],
                                 start=True, stop=True)
                res_sb = at.tile([C, BC], dt.float32, tag="res")
                nc.scalar.activation(
                    out=res_sb[:, :], in_=pso[:, :], func=AF.Identity,
                    bias=b2_sb[:, 0:1], scale=1.0)
                nc.sync.dma_start(out=res_o[:, :], in_=res_sb[:, :])

    return nc


def _legalize_sync(nc):
    """This walrus build only accepts ONE sync wait (and one update) per
    instruction (NEURON_ISA_TPB_EVENTS has a single wait slot). Tile emits
    multi-wait sync_info; split the excess onto NOPs inserted just before
    (waits) / after (updates) the offending instruction on the same engine."""
    import concourse.mybir as mybir

    nid = [0]

    def mknop(engine, waits, updates, debug):
        nid[0] += 1
        return mybir.InstNoOp(
            name=f"I-syncfix-{nid[0]}", engine=engine, ins=[], outs=[],
            debug=debug,
            sync_info=mybir.SyncInfo(on_wait=waits, on_update=updates))

    def fix_block(bb):
        new = []
        for inst in bb.instructions:
            si = getattr(inst, "sync_info", None)
            ow = list(si.on_wait) if si is not None and si.on_wait else []
            ou = list(si.on_update) if si is not None and si.on_update else []
            pre = []
            post = []
            if len(ow) > 1:
                for w in ow[:-1]:
                    pre.append(mknop(inst.engine, [w], [], inst.debug))
                ow = ow[-1:]
            if len(ou) > 1:
                for u in ou[1:]:
                    post.append(mknop(inst.engine, [], [u], inst.debug))
                ou = ou[:1]
            if pre or post:
                inst.sync_info = mybir.SyncInfo(on_wait=ow, on_update=ou)
            new.extend(pre)
            new.append(inst)
            new.extend(post)
        bb.instructions[:] = new

    def walk(block):
        for bb in block:
            fix_block(bb)

    for f in nc.m.functions:
        for bb in f.blocks:
            fix_block(bb)
    return nc


def _prep(inputs):
    toks = np.asarray(inputs["inputs"]).astype(np.int32)       # [B, T]
    lens = np.asarray(inputs["seq_lengths"]).astype(np.int64)  # [B]
    pad = int(np.asarray(inputs["pad_token"]))
    emb = np.asarray(inputs["emb"], dtype=F32)
    W_ih = np.asarray(inputs["W_ih"], dtype=F32)
    b_ih = np.asarray(inputs["b_ih"], dtype=F32)
    W_hh = np.asarray(inputs["W_hh"], dtype=F32)
    b_hh = np.asarray(inputs["b_hh"], dtype=F32)
    Wa = np.asarray(inputs["Wa"], dtype=F32)
    # ba cancels in softmax; W1/b1/W2/b2 used below
    W1 = np.asarray(inputs["W1"], dtype=F32)
    b1 = np.asarray(inputs["b1"], dtype=F32)
    W2 = np.asarray(inputs["W2"], dtype=F32)
    b2 = np.asarray(inputs["b2"], dtype=F32)

    emb_bf = np.ascontiguousarray(emb.astype(BF16))
    whhT_p = np.ascontiguousarray(
        W_hh.reshape(MT, 128, MT, 128).transpose(3, 2, 0, 1).reshape(128, MT * H)
        .astype(BF16))
    wihT_p = np.ascontiguousarray(
        W_ih.reshape(MT, 128, ET, 128).transpose(3, 2, 0, 1).reshape(128, ET * H)
        .astype(BF16))
    wa_p = np.ascontiguousarray(
        Wa.reshape(MT, 128, MT, 128).transpose(1, 0, 2, 3).reshape(128, MT * H)
        .astype(BF16))
    w1T_p = np.ascontiguousarray(
        W1.reshape(64, MT, 128).transpose(2, 1, 0).reshape(128, MT * 64)
        .astype(BF16))
    w2T_p = np.ascontiguousarray(W2.T.astype(BF16))
    ball_p = np.ascontiguousarray((b_ih + b_hh).reshape(MT, 128).T.astype(F32))
    b1_p = np.ascontiguousarray(b1.reshape(64, 1).astype(F32))
    b2_p = np.ascontiguousarray(b2.reshape(C, 1).astype(F32))

    in_maps = []
    for c in range(NCORES):
        tb = toks[c * BC:(c + 1) * BC]          # [8, T]
        ln = lens[c * BC:(c + 1) * BC]          # [8]
        # token order r = t*8 + b, laid out as [128, RT] with r = i*128+p
        flat = np.ascontiguousarray(tb.T).reshape(-1)          # [T*BC]
        tok_p = np.ascontiguousarray(
            flat.reshape(RT, 128).T.astype(np.int32))          # [128, RT]
        am = np.where(tb == pad, -1e6, 0.0).astype(F32)        # [8, T]
        am_p = np.ascontiguousarray(am.reshape(1, BC * T))
        sel = np.zeros((BC, T), dtype=F32)
        sel[np.arange(BC), np.clip(ln - 1, 0, T - 1)] = 1.0
        sel_p = np.ascontiguousarray(sel.reshape(1, BC * T).astype(BF16))
        in_maps.append({
            "emb_bf": emb_bf, "tok_idx": tok_p,
            "whhT": whhT_p, "wihT": wihT_p, "wa_i": wa_p,
            "w1T": w1T_p, "w2T": w2T_p,
            "ball": ball_p, "b1_i": b1_p, "b2_i": b2_p,
            "amask": am_p, "sel_i": sel_p,
        })
    return in_maps


def kernel(**inputs):
    from concourse.bass_utils import run_bass_kernel_spmd

    in_maps = _prep(inputs)
    nc = _build()
    _legalize_sync(nc)
    r = run_bass_kernel_spmd(nc, in_maps, core_ids=list(range(NCORES)))
    if r.exec_time_ns is not None:
        print(f"HW exec time: {r.exec_time_ns} ns")
        if r.instructions_and_trace is not None:
            print(f"trace: {r.instructions_and_trace[1]}")
    out = np.zeros((B, C), dtype=F32)
    for c in range(NCORES):
        out[c * BC:(c + 1) * BC] = r.results[c]["res"].T
    return out

